# revision 11
# baseline (speedup 1.0000x reference)
import os
import sys

for _p in ("/opt/trn_rl_repo", "/root/.axon_site/_ro/trn_rl_repo"):
    if os.path.isdir(_p) and _p not in sys.path:
        sys.path.insert(0, _p)

from contextlib import ExitStack

import numpy as np

from concourse import bacc, mybir, tile
from concourse.bass_utils import run_bass_kernel_spmd

B, N, M, C = 1024, 1024, 64, 512
L = 3 * M + 6  # 198
NCORES = 8
BS = B // NCORES  # 128 rows per core == SBUF partitions
NT = 64
NCH = N // NT  # 16
EPS = 1e-8

F32 = mybir.dt.float32
ALU = mybir.AluOpType
ACTF = mybir.ActivationFunctionType
AX = mybir.AxisListType

_CACHE = {}


def _build():
    if "nc" in _CACHE:
        return _CACHE["nc"]
    nc = bacc.Bacc("TRN2", debug=False, target_bir_lowering=False, num_devices=NCORES)

    ew = nc.dram_tensor("ew", (C, BS + L), F32, kind="ExternalInput").ap()
    blin = nc.dram_tensor("blin", (BS, L), F32, kind="ExternalInput").ap()
    wold = nc.dram_tensor("wold", (BS, N), F32, kind="ExternalInput").ap()
    mem = nc.dram_tensor("mem", (BS, N, M), F32, kind="ExternalInput").ap()
    w_out = nc.dram_tensor("w_out", (BS, N), F32, kind="ExternalOutput").ap()
    nmem = nc.dram_tensor("nmem", (BS, N, M), F32, kind="ExternalOutput").ap()

    with tile.TileContext(nc) as tc, ExitStack() as ctx:
        const = ctx.enter_context(tc.tile_pool(name="const", bufs=1))
        work = ctx.enter_context(tc.tile_pool(name="work", bufs=1))
        mp = ctx.enter_context(tc.tile_pool(name="mp", bufs=2))
        sp = ctx.enter_context(tc.tile_pool(name="sp", bufs=2))
        op = ctx.enter_context(tc.tile_pool(name="op", bufs=3))
        pp = ctx.enter_context(tc.tile_pool(name="pp", bufs=1, space="PSUM"))

        # ---------- small loads ----------
        ew_sb = const.tile([128, C // 128, BS + L], F32)
        blin_sb = const.tile([BS, L], F32)
        wold_sb = const.tile([BS, N], F32)
        nc.sync.dma_start(ew_sb[:], ew.rearrange("(a p) c -> p a c", p=128))
        nc.sync.dma_start(blin_sb[:], blin[:])
        nc.sync.dma_start(wold_sb[:], wold[:])

        # ---------- linear: lin = emb @ W_lin.T + b ----------
        lin_ps = pp.tile([BS, L], F32)
        kchunks = C // 128
        for ki in range(kchunks):
            nc.tensor.matmul(
                lin_ps[:],
                ew_sb[:, ki, 0:BS],
                ew_sb[:, ki, BS : BS + L],
                start=(ki == 0),
                stop=(ki == kchunks - 1),
            )
        lin = work.tile([BS, L], F32)
        nc.vector.tensor_tensor(lin[:], lin_ps[:], blin_sb[:], ALU.add)

        # ---------- head params ----------
        # softplus(x) = relu(x) + ln(1 + exp(-|x|)), batched for beta & gamma cols
        sp_in = work.tile([BS, 2], F32)
        nc.vector.tensor_copy(sp_in[:, 0:1], lin[:, M : M + 1])
        nc.vector.tensor_copy(sp_in[:, 1:2], lin[:, M + 5 : M + 6])
        sp_ax = work.tile([BS, 2], F32)
        nc.scalar.activation(sp_ax[:], sp_in[:], ACTF.Abs)
        sp_en = work.tile([BS, 2], F32)
        nc.scalar.activation(sp_en[:], sp_ax[:], ACTF.Exp, scale=-1.0)
        nc.vector.tensor_scalar_add(sp_en[:], sp_en[:], 1.0)
        sp_ln = work.tile([BS, 2], F32)
        nc.scalar.activation(sp_ln[:], sp_en[:], ACTF.Ln)
        sp_re = work.tile([BS, 2], F32)
        nc.scalar.activation(sp_re[:], sp_in[:], ACTF.Relu)
        sp_out = work.tile([BS, 2], F32)
        nc.vector.tensor_tensor(sp_out[:], sp_re[:], sp_ln[:], ALU.add)
        beta = sp_out[:, 0:1]
        g = work.tile([BS, 1], F32)
        nc.scalar.activation(g[:], lin[:, M + 1 : M + 2], ACTF.Sigmoid)
        gm1 = work.tile([BS, 1], F32)  # 1-g = sigmoid(-x)
        nc.scalar.activation(gm1[:], lin[:, M + 1 : M + 2], ACTF.Sigmoid, scale=-1.0)
        gamma = work.tile([BS, 1], F32)
        nc.vector.tensor_scalar_add(gamma[:], sp_out[:, 1:2], 1.0)
        e_sb = work.tile([BS, M], F32)
        nc.scalar.activation(e_sb[:], lin[:, M + 6 : 2 * M + 6], ACTF.Sigmoid)

        # s = softmax(lin[:, M+2:M+5]) over the 3 taps
        smax_neg = work.tile([BS, 1], F32)
        nc.vector.tensor_reduce(
            smax_neg[:], lin[:, M + 2 : M + 5], axis=AX.X, op=ALU.max, negate=True
        )
        s_exp = work.tile([BS, 3], F32)
        s_sum = work.tile([BS, 1], F32)
        nc.scalar.activation(
            s_exp[:], lin[:, M + 2 : M + 5], ACTF.Exp, bias=smax_neg[:], accum_out=s_sum[:]
        )
        s_rcp = work.tile([BS, 1], F32)
        nc.vector.reciprocal(s_rcp[:], s_sum[:])
        s_sb = work.tile([BS, 3], F32)
        nc.vector.tensor_scalar_mul(s_sb[:], s_exp[:], s_rcp[:])

        # kn = k / (||k|| + eps)
        ksq = work.tile([BS, M], F32)
        ksq_sum = work.tile([BS, 1], F32)
        nc.scalar.activation(ksq[:], lin[:, 0:M], ACTF.Square, accum_out=ksq_sum[:])
        knorm = work.tile([BS, 1], F32)
        nc.scalar.activation(knorm[:], ksq_sum[:], ACTF.Sqrt)
        nc.vector.tensor_scalar_add(knorm[:], knorm[:], EPS)
        k_rcp = work.tile([BS, 1], F32)
        nc.vector.reciprocal(k_rcp[:], knorm[:])
        kn = work.tile([BS, M], F32)
        nc.vector.tensor_scalar_mul(kn[:], lin[:, 0:M], k_rcp[:])

        # ---------- pass A: dot[b,n] = <mem[b,n,:], kn[b,:]>, sq[b,n] = <mem,mem> ----------
        dot_f = work.tile([BS, N], F32)
        sq_f = work.tile([BS, N], F32)
        kn_b = kn[:].unsqueeze(1).broadcast_to([BS, NT, M])
        for i in range(NCH):
            mem_t = mp.tile([BS, NT, M], F32)
            nc.sync.dma_start(mem_t[:], mem[:, i * NT : (i + 1) * NT, :])
            prod = sp.tile([BS, NT, M], F32)
            nc.gpsimd.tensor_tensor(prod[:], mem_t[:], kn_b, ALU.mult)
            nc.vector.tensor_reduce(
                dot_f[:, i * NT : (i + 1) * NT], prod[:], axis=AX.X, op=ALU.add
            )
            nc.scalar.activation(prod[:], mem_t[:], ACTF.Square)
            nc.vector.tensor_reduce(
                sq_f[:, i * NT : (i + 1) * NT], prod[:], axis=AX.X, op=ALU.add
            )

        # ---------- content weights ----------
        nrm = work.tile([BS, N], F32)
        nc.scalar.activation(nrm[:], sq_f[:], ACTF.Sqrt)
        nc.vector.tensor_scalar_add(nrm[:], nrm[:], EPS)
        rcp = work.tile([BS, N], F32)
        nc.vector.reciprocal(rcp[:], nrm[:])
        sc = work.tile([BS, N], F32)
        nc.vector.scalar_tensor_tensor(sc[:], dot_f[:], beta[:], rcp[:], ALU.mult, ALU.mult)

        cmax_neg = work.tile([BS, 1], F32)
        nc.vector.tensor_reduce(cmax_neg[:], sc[:], axis=AX.X, op=ALU.max, negate=True)
        exp_t = work.tile([BS, N], F32)
        esum = work.tile([BS, 1], F32)
        nc.scalar.activation(
            exp_t[:], sc[:], ACTF.Exp, bias=cmax_neg[:], accum_out=esum[:]
        )
        ercp = work.tile([BS, 1], F32)
        nc.vector.reciprocal(ercp[:], esum[:])
        gs = work.tile([BS, 1], F32)  # g / esum
        nc.vector.tensor_tensor(gs[:], g[:], ercp[:], ALU.mult)

        # w_g = g*softmax + (1-g)*w_old, written into halo buffer positions 1..N
        w_gh = work.tile([BS, N + 2], F32)
        tmp_w = work.tile([BS, N], F32)
        nc.vector.tensor_scalar_mul(tmp_w[:], wold_sb[:], gm1[:])
        nc.vector.scalar_tensor_tensor(
            w_gh[:, 1 : N + 1], exp_t[:], gs[:], tmp_w[:], ALU.mult, ALU.add
        )
        # circular halo: pos0 <- w_g[N-1], pos N+1 <- w_g[0]
        nc.scalar.activation(w_gh[:, 0:1], w_gh[:, N : N + 1], ACTF.Copy)
        nc.scalar.activation(w_gh[:, N + 1 : N + 2], w_gh[:, 1:2], ACTF.Copy)

        # 3-tap circular conv
        w_s = work.tile([BS, N], F32)
        nc.vector.tensor_scalar_mul(w_s[:], w_gh[:, 0:N], s_sb[:, 0:1])
        nc.vector.scalar_tensor_tensor(
            w_s[:], w_gh[:, 1 : N + 1], s_sb[:, 1:2], w_s[:], ALU.mult, ALU.add
        )
        nc.vector.scalar_tensor_tensor(
            w_s[:], w_gh[:, 2 : N + 2], s_sb[:, 2:3], w_s[:], ALU.mult, ALU.add
        )

        # sharpen: W = (w_s+eps)^gamma / sum
        eps_col = work.tile([BS, 1], F32)
        nc.gpsimd.memset(eps_col[:], EPS)
        ln_t = work.tile([BS, N], F32)
        nc.scalar.activation(ln_t[:], w_s[:], ACTF.Ln, bias=eps_col[:])
        w_pow = work.tile([BS, N], F32)
        psum_s = work.tile([BS, 1], F32)
        nc.scalar.activation(
            w_pow[:], ln_t[:], ACTF.Exp, scale=gamma[:], accum_out=psum_s[:]
        )
        prcp = work.tile([BS, 1], F32)
        nc.vector.reciprocal(prcp[:], psum_s[:])
        W_sb = work.tile([BS, N], F32)
        nc.vector.tensor_scalar_mul(W_sb[:], w_pow[:], prcp[:])
        nc.sync.dma_start(w_out[:], W_sb[:])

        # ---------- pass B: nmem = mem - W*(mem*e - a) ----------
        e_b = e_sb[:].unsqueeze(1).broadcast_to([BS, NT, M])
        a_b = lin[:, 2 * M + 6 : 3 * M + 6].unsqueeze(1).broadcast_to([BS, NT, M])
        for i in range(NCH):
            mem_t = mp.tile([BS, NT, M], F32)
            nc.sync.dma_start(mem_t[:], mem[:, i * NT : (i + 1) * NT, :])
            t1 = op.tile([BS, NT, M], F32)
            nc.vector.tensor_tensor(t1[:], mem_t[:], e_b, ALU.mult)
            nc.gpsimd.tensor_tensor(t1[:], t1[:], a_b, ALU.subtract)
            w_b = W_sb[:, i * NT : (i + 1) * NT].unsqueeze(2).broadcast_to([BS, NT, M])
            nc.vector.tensor_tensor(t1[:], t1[:], w_b, ALU.mult)
            nc.vector.tensor_tensor(t1[:], mem_t[:], t1[:], ALU.subtract)
            nc.sync.dma_start(nmem[:, i * NT : (i + 1) * NT, :], t1[:])

    nc.finalize()
    _CACHE["nc"] = nc
    return nc


def kernel(rnn_embeddings, W_old, memory, W_lin, b_lin):
    rnn_embeddings = np.asarray(rnn_embeddings, dtype=np.float32)
    W_old = np.asarray(W_old, dtype=np.float32)
    memory = np.asarray(memory, dtype=np.float32)
    W_lin = np.asarray(W_lin, dtype=np.float32)
    b_lin = np.asarray(b_lin, dtype=np.float32)

    nc = _build()
    wlinT = W_lin.T  # [C, L]
    blin = np.ascontiguousarray(np.broadcast_to(b_lin[None, :], (BS, L)))
    in_maps = []
    for c in range(NCORES):
        sl = slice(c * BS, (c + 1) * BS)
        in_maps.append(
            {
                "ew": np.ascontiguousarray(
                    np.concatenate([rnn_embeddings[sl].T, wlinT], axis=1)
                ),
                "blin": blin,
                "wold": np.ascontiguousarray(W_old[sl]),
                "mem": np.ascontiguousarray(memory[sl]),
            }
        )
    res = run_bass_kernel_spmd(nc, in_maps, list(range(NCORES))).results
    W = np.concatenate([res[c]["w_out"] for c in range(NCORES)], axis=0)
    new_memory = np.concatenate([res[c]["nmem"] for c in range(NCORES)], axis=0)
    return W.astype(np.float32), new_memory.astype(np.float32)


# revision 21
# speedup vs baseline: 114.7500x; 114.7500x over previous
import os
import sys

for _p in ("/opt/trn_rl_repo", "/root/.axon_site/_ro/trn_rl_repo"):
    if os.path.isdir(_p) and _p not in sys.path:
        sys.path.insert(0, _p)

from contextlib import ExitStack

import numpy as np

from concourse import bacc, mybir, tile
from concourse.bass_utils import run_bass_kernel_spmd

B, N, M, C = 1024, 1024, 64, 512
L = 3 * M + 6  # 198
NCORES = 8
BS = B // NCORES  # 128 rows per core == SBUF partitions
NT = 64
NCH = N // NT  # 16
NCACHE = 6  # chunks 10..15 stay resident in SBUF between pass A and pass B
EPS = 1e-8

F32 = mybir.dt.float32
ALU = mybir.AluOpType
ACTF = mybir.ActivationFunctionType
AX = mybir.AxisListType

_CACHE = {}


def _build(reps=1):
    key = ("nc", reps)
    if key in _CACHE:
        return _CACHE[key]
    nc = bacc.Bacc("TRN2", debug=False, target_bir_lowering=False, num_devices=NCORES)

    ew = nc.dram_tensor("ew", (C, BS + L), F32, kind="ExternalInput").ap()
    blin = nc.dram_tensor("blin", (BS, L), F32, kind="ExternalInput").ap()
    wold = nc.dram_tensor("wold", (BS, N), F32, kind="ExternalInput").ap()
    mem = nc.dram_tensor("mem", (BS, N, M), F32, kind="ExternalInput").ap()
    w_out = nc.dram_tensor("w_out", (BS, N), F32, kind="ExternalOutput").ap()
    nmem = nc.dram_tensor("nmem", (BS, N, M), F32, kind="ExternalOutput").ap()

    with tile.TileContext(nc) as tc, ExitStack() as ctx:
        const = ctx.enter_context(tc.tile_pool(name="const", bufs=1))
        work = ctx.enter_context(tc.tile_pool(name="work", bufs=1))
        scr = ctx.enter_context(tc.tile_pool(name="scr", bufs=3))
        mp = ctx.enter_context(tc.tile_pool(name="mp", bufs=7))
        pp = ctx.enter_context(tc.tile_pool(name="pp", bufs=1, space="PSUM"))

        def body():
            # ---------- small loads ----------
            ew_sb = scr.tile([128, C // 128, BS + L], F32, tag="scr")
            blin_sb = const.tile([BS, L], F32)
            wold_sb = const.tile([BS, N], F32)
            nc.sync.dma_start(ew_sb[:], ew.rearrange("(a p) c -> p a c", p=128))
            nc.sync.dma_start(blin_sb[:], blin[:])
            nc.scalar.dma_start(wold_sb[:], wold[:])

            # big rotating roles:
            #   bufA: dot_f -> exp_t -> w_s -> w_pow
            #   bufB: sq_f -> rcp
            #   bufC: nrm -> sc -> ln_t
            #   bufD: W_sb
            #   bufE: w_gh (halo)
            bufA = work.tile([BS, N], F32)
            bufB = work.tile([BS, N], F32)
            bufC = work.tile([BS, N], F32)
            bufD = work.tile([BS, N], F32)
            bufE = work.tile([BS, N + 2], F32)

            # ---------- linear: lin = emb @ W_lin.T + b ----------
            lin_ps = pp.tile([BS, L], F32, tag="ps")
            kchunks = C // 128
            for ki in range(kchunks):
                nc.tensor.matmul(
                    lin_ps[:],
                    ew_sb[:, ki, 0:BS],
                    ew_sb[:, ki, BS : BS + L],
                    start=(ki == 0),
                    stop=(ki == kchunks - 1),
                )
            lin = work.tile([BS, L], F32)
            nc.vector.tensor_tensor(lin[:], lin_ps[:], blin_sb[:], ALU.add)

            # ---------- head params ----------
            # softplus(x) = relu(x) + ln(1 + exp(-|x|)) for beta & gamma cols
            sp_in = work.tile([BS, 2], F32)
            nc.vector.tensor_copy(sp_in[:, 0:1], lin[:, M : M + 1])
            nc.vector.tensor_copy(sp_in[:, 1:2], lin[:, M + 5 : M + 6])
            sp_ax = work.tile([BS, 2], F32)
            nc.scalar.activation(sp_ax[:], sp_in[:], ACTF.Abs)
            sp_en = work.tile([BS, 2], F32)
            nc.scalar.activation(sp_en[:], sp_ax[:], ACTF.Exp, scale=-1.0)
            nc.vector.tensor_scalar_add(sp_en[:], sp_en[:], 1.0)
            sp_ln = work.tile([BS, 2], F32)
            nc.scalar.activation(sp_ln[:], sp_en[:], ACTF.Ln)
            sp_re = work.tile([BS, 2], F32)
            nc.scalar.activation(sp_re[:], sp_in[:], ACTF.Relu)
            sp_out = work.tile([BS, 2], F32)
            nc.vector.tensor_tensor(sp_out[:], sp_re[:], sp_ln[:], ALU.add)
            beta = sp_out[:, 0:1]
            g = work.tile([BS, 1], F32)
            nc.scalar.activation(g[:], lin[:, M + 1 : M + 2], ACTF.Sigmoid)
            gm1 = work.tile([BS, 1], F32)  # 1-g = sigmoid(-x)
            nc.scalar.activation(gm1[:], lin[:, M + 1 : M + 2], ACTF.Sigmoid, scale=-1.0)
            gamma = work.tile([BS, 1], F32)
            nc.vector.tensor_scalar_add(gamma[:], sp_out[:, 1:2], 1.0)
            e_sb = work.tile([BS, M], F32)
            nc.scalar.activation(e_sb[:], lin[:, M + 6 : 2 * M + 6], ACTF.Sigmoid)

            # s = softmax(lin[:, M+2:M+5]) over the 3 taps
            smax_neg = work.tile([BS, 1], F32)
            nc.vector.tensor_reduce(
                smax_neg[:], lin[:, M + 2 : M + 5], axis=AX.X, op=ALU.max, negate=True
            )
            s_exp = work.tile([BS, 3], F32)
            s_sum = work.tile([BS, 1], F32)
            nc.scalar.activation(
                s_exp[:], lin[:, M + 2 : M + 5], ACTF.Exp, bias=smax_neg[:],
                accum_out=s_sum[:],
            )
            s_rcp = work.tile([BS, 1], F32)
            nc.vector.reciprocal(s_rcp[:], s_sum[:])
            s_sb = work.tile([BS, 3], F32)
            nc.vector.tensor_scalar_mul(s_sb[:], s_exp[:], s_rcp[:])

            # kn = k / (||k|| + eps)
            ksq = work.tile([BS, M], F32)
            ksq_sum = work.tile([BS, 1], F32)
            nc.scalar.activation(ksq[:], lin[:, 0:M], ACTF.Square, accum_out=ksq_sum[:])
            knorm = work.tile([BS, 1], F32)
            nc.scalar.activation(knorm[:], ksq_sum[:], ACTF.Sqrt)
            nc.vector.tensor_scalar_add(knorm[:], knorm[:], EPS)
            k_rcp = work.tile([BS, 1], F32)
            nc.vector.reciprocal(k_rcp[:], knorm[:])
            kn = work.tile([BS, M], F32)
            nc.vector.tensor_scalar_mul(kn[:], lin[:, 0:M], k_rcp[:])

            # ---------- pass A: dot[b,n] = <mem[b,n,:], kn[b,:]>, sq = <mem,mem> ----------
            kn_b = kn[:].unsqueeze(1).broadcast_to([BS, NT, M])
            mem_tiles = []
            for i in range(NCH):
                eng = nc.sync if i % 2 == 0 else nc.scalar
                mem_t = mp.tile([BS, NT, M], F32)
                eng.dma_start(mem_t[:], mem[:, i * NT : (i + 1) * NT, :])
                mem_tiles.append(mem_t)
                prod = scr.tile([BS, NT, M], F32, tag="scr")
                nc.gpsimd.tensor_tensor(prod[:], mem_t[:], kn_b, ALU.mult)
                nc.vector.tensor_reduce(
                    bufA[:, i * NT : (i + 1) * NT], prod[:], axis=AX.X, op=ALU.add
                )
                # square scratch lives in PSUM (lin_ps is dead by now) so the
                # ACT square doesn't serialize against the prod reduce
                sq_t = pp.tile([BS, NT, M], F32, tag="ps")
                nc.scalar.activation(sq_t[:], mem_t[:], ACTF.Square)
                nc.vector.tensor_reduce(
                    bufB[:, i * NT : (i + 1) * NT], sq_t[:], axis=AX.X, op=ALU.add
                )

            # ---------- content weights ----------
            nc.scalar.activation(bufC[:], bufB[:], ACTF.Sqrt)  # nrm
            nc.vector.tensor_scalar_add(bufC[:], bufC[:], EPS)
            nc.vector.reciprocal(bufB[:], bufC[:])  # rcp
            nc.vector.scalar_tensor_tensor(  # sc = beta * dot * rcp
                bufC[:], bufA[:], beta[:], bufB[:], ALU.mult, ALU.mult
            )

            cmax_neg = work.tile([BS, 1], F32)
            nc.vector.tensor_reduce(
                cmax_neg[:], bufC[:], axis=AX.X, op=ALU.max, negate=True
            )
            esum = work.tile([BS, 1], F32)
            nc.scalar.activation(  # exp_t
                bufA[:], bufC[:], ACTF.Exp, bias=cmax_neg[:], accum_out=esum[:]
            )
            ercp = work.tile([BS, 1], F32)
            nc.vector.reciprocal(ercp[:], esum[:])
            gs = work.tile([BS, 1], F32)  # g / esum
            nc.vector.tensor_tensor(gs[:], g[:], ercp[:], ALU.mult)

            # w_g = g*softmax + (1-g)*w_old, into halo buffer positions 1..N
            nc.vector.tensor_scalar_mul(wold_sb[:], wold_sb[:], gm1[:])
            nc.vector.scalar_tensor_tensor(
                bufE[:, 1 : N + 1], bufA[:], gs[:], wold_sb[:], ALU.mult, ALU.add
            )
            # circular halo: pos0 <- w_g[N-1], pos N+1 <- w_g[0]
            nc.scalar.activation(bufE[:, 0:1], bufE[:, N : N + 1], ACTF.Copy)
            nc.scalar.activation(bufE[:, N + 1 : N + 2], bufE[:, 1:2], ACTF.Copy)

            # 3-tap circular conv -> w_s in bufA
            nc.vector.tensor_scalar_mul(bufA[:], bufE[:, 0:N], s_sb[:, 0:1])
            nc.vector.scalar_tensor_tensor(
                bufA[:], bufE[:, 1 : N + 1], s_sb[:, 1:2], bufA[:], ALU.mult, ALU.add
            )
            nc.vector.scalar_tensor_tensor(
                bufA[:], bufE[:, 2 : N + 2], s_sb[:, 2:3], bufA[:], ALU.mult, ALU.add
            )

            # sharpen: W = (w_s+eps)^gamma / sum
            eps_col = work.tile([BS, 1], F32)
            nc.gpsimd.memset(eps_col[:], EPS)
            nc.scalar.activation(bufC[:], bufA[:], ACTF.Ln, bias=eps_col[:])  # ln_t
            psum_s = work.tile([BS, 1], F32)
            nc.scalar.activation(  # w_pow
                bufA[:], bufC[:], ACTF.Exp, scale=gamma[:], accum_out=psum_s[:]
            )
            prcp = work.tile([BS, 1], F32)
            nc.vector.reciprocal(prcp[:], psum_s[:])
            nc.vector.tensor_scalar_mul(bufD[:], bufA[:], prcp[:])  # W_sb
            nc.sync.dma_start(w_out[:], bufD[:])

            # ---------- pass B: nmem = mem - W*(mem*e - a) ----------
            # last NCACHE chunks are still SBUF-resident from pass A; the rest
            # are re-loaded (first freed slot lets prefetch overlap the head)
            e_b = e_sb[:].unsqueeze(1).broadcast_to([BS, NT, M])
            a_b = lin[:, 2 * M + 6 : 3 * M + 6].unsqueeze(1).broadcast_to([BS, NT, M])
            order = list(range(NCH - NCACHE, NCH)) + list(range(0, NCH - NCACHE))
            for j, i in enumerate(order):
                if j < NCACHE:
                    mt = mem_tiles[i]
                else:
                    eng = nc.sync if i % 2 == 0 else nc.scalar
                    mem_t = mp.tile([BS, NT, M], F32)
                    eng.dma_start(mem_t[:], mem[:, i * NT : (i + 1) * NT, :])
                    mt = mem_t
                t1 = scr.tile([BS, NT, M], F32, tag="scr")
                nc.vector.tensor_tensor(t1[:], mt[:], e_b, ALU.mult)
                nc.gpsimd.tensor_tensor(t1[:], t1[:], a_b, ALU.subtract)
                w_b = (
                    bufD[:, i * NT : (i + 1) * NT]
                    .unsqueeze(2)
                    .broadcast_to([BS, NT, M])
                )
                mul_eng = nc.gpsimd if j % 3 == 2 else nc.vector
                mul_eng.tensor_tensor(t1[:], t1[:], w_b, ALU.mult)
                nc.vector.tensor_tensor(t1[:], mt[:], t1[:], ALU.subtract)
                st = nc.scalar if i % 2 == 0 else nc.sync
                st.dma_start(nmem[:, i * NT : (i + 1) * NT, :], t1[:])

        for _rep in range(reps):
            body()

    nc.finalize()
    _CACHE[key] = nc
    return nc


def kernel(rnn_embeddings, W_old, memory, W_lin, b_lin):
    rnn_embeddings = np.asarray(rnn_embeddings, dtype=np.float32)
    W_old = np.asarray(W_old, dtype=np.float32)
    memory = np.asarray(memory, dtype=np.float32)
    W_lin = np.asarray(W_lin, dtype=np.float32)
    b_lin = np.asarray(b_lin, dtype=np.float32)

    nc = _build()
    wlinT = W_lin.T  # [C, L]
    blin = np.ascontiguousarray(np.broadcast_to(b_lin[None, :], (BS, L)))
    in_maps = []
    for c in range(NCORES):
        sl = slice(c * BS, (c + 1) * BS)
        in_maps.append(
            {
                "ew": np.ascontiguousarray(
                    np.concatenate([rnn_embeddings[sl].T, wlinT], axis=1)
                ),
                "blin": blin,
                "wold": np.ascontiguousarray(W_old[sl]),
                "mem": np.ascontiguousarray(memory[sl]),
            }
        )
    res = run_bass_kernel_spmd(nc, in_maps, list(range(NCORES))).results
    W = np.concatenate([res[c]["w_out"] for c in range(NCORES)], axis=0)
    new_memory = np.concatenate([res[c]["nmem"] for c in range(NCORES)], axis=0)
    return W.astype(np.float32), new_memory.astype(np.float32)
